# revision 2
# baseline (speedup 1.0000x reference)
"""Causal RBF (non-softmax) attention on 8 Trainium2 NeuronCores.

Problem: q,k,v [B=2, H=16, N=2048, D=128] f32.
  logits = 2s*q@k^T - s*||q||^2 - s*||k||^2   (s = 1/sqrt(D))
  p = exp(logits) with causal mask; out = p @ v      (no softmax normalization)

Sharding: B*H = 32 heads -> 4 heads per core, fully independent.

Algebra used to make the device kernel cheap:
  out[m,:] = eq[m] * sum_{n<=m} exp(2s*q_m.k_n) * (ek[n]*v[n,:])
  with eq[m] = exp(-s*||q_m||^2), ek[n] = exp(-s*||k_n||^2).
The host folds 2s into qT, ek into v, and applies eq to the output rows, so
the device computes only:  ST = KT^T.T @ QT blocks -> Exp -> mask -> @ V'.

Device layouts (per head):
  qT [128(d), 2048(m)]  (host-transposed, scaled by 2s)
  kT [128(d), 2048(n)]  (host-transposed)
  v' [2048(n), 128(d)]  (ek-scaled, natural)
Output is written transposed, OT [128(d), 2048(m)]; host transposes back.

Matmuls run as float32r (full PE rate at free-dim 512). PSUM: ST group tile
[128, 4, 512] double-buffered = all 8 banks; PV matmuls accumulate into bank 0
of their own group's tile (legal: exp already consumed it), and DVE adds the
per-group partial into an SBUF accumulator.
"""

import math
import os
import sys

import numpy as np

sys.path.insert(0, "/opt/trn_rl_repo")

import concourse.bass as bass
import concourse.mybir as mybir
import concourse.tile as tile
from concourse import bacc, bass_utils

F32 = mybir.dt.float32
F32R = mybir.dt.float32r
EXP = mybir.ActivationFunctionType.Exp

B, H, N, D = 2, 16, 2048, 128
SM = 1.0 / math.sqrt(D)
P = 128
NCORES = 8
HPC = (B * H) // NCORES  # heads per core
MW = 512                 # m (query) super-tile width
G = 4                    # k-blocks per group (4 x 512 f32 = 4 PSUM banks)


def _emit_body(tc, qt, kt, v, mask, out, hpc, n):
    nc = tc.nc
    nb = n // P     # 128-wide k blocks
    mi = n // MW    # query super tiles
    from contextlib import ExitStack

    with ExitStack() as ctx:
        const = ctx.enter_context(tc.tile_pool(name="const", bufs=1))
        qk_pool = ctx.enter_context(tc.tile_pool(name="qk", bufs=2))
        v_pool = ctx.enter_context(tc.tile_pool(name="vp", bufs=2))
        st_pool = ctx.enter_context(tc.tile_pool(name="st", bufs=2, space="PSUM"))
        pt_pool = ctx.enter_context(tc.tile_pool(name="pt", bufs=3))
        ot_pool = ctx.enter_context(tc.tile_pool(name="ot", bufs=2))

        mask_sb = const.tile([P, G, MW], F32R)
        nc.sync.dma_start(mask_sb[:], mask[:])

        for h in range(hpc):
            qt_sb = qk_pool.tile([P, n], F32R, tag="qt")
            kt_sb = qk_pool.tile([P, n], F32R, tag="kt")
            v_sb = v_pool.tile([P, nb, P], F32R, tag="v")
            nc.sync.dma_start(qt_sb[:], qt[h])
            nc.sync.dma_start(kt_sb[:], kt[h])
            nc.sync.dma_start(v_sb[:], v[h].rearrange("(nb p) d -> p nb d", p=P))

            for i in range(mi):
                ngroups = (i + 1) * (MW // (G * P))  # groups of G k-blocks to cover (i+1)*MW keys
                ot_sb = ot_pool.tile([P, MW], F32, tag="ot")
                for g in range(ngroups):
                    st = st_pool.tile([P, G, MW], F32, tag="st")
                    for t in range(G):
                        j = G * g + t
                        nc.tensor.matmul(
                            st[:, t, :],
                            lhsT=kt_sb[:, j * P : (j + 1) * P],
                            rhs=qt_sb[:, i * MW : (i + 1) * MW],
                            start=True,
                            stop=True,
                        )
                    pt = pt_pool.tile([P, G, MW], F32R, tag="pt")
                    nc.scalar.activation(pt[:], st[:], EXP)
                    if g == ngroups - 1:
                        # diagonal group: causal mask (zero where key > query)
                        nc.vector.tensor_mul(pt[:], pt[:], mask_sb[:])
                    for t in range(G):
                        j = G * g + t
                        nc.tensor.matmul(
                            st[:, 0, :],
                            lhsT=v_sb[:, j, :],
                            rhs=pt[:, t, :],
                            start=(t == 0),
                            stop=(t == G - 1),
                        )
                    if g == 0:
                        nc.vector.tensor_copy(ot_sb[:], st[:, 0, :])
                    else:
                        nc.vector.tensor_add(ot_sb[:], ot_sb[:], st[:, 0, :])
                nc.sync.dma_start(out[h, :, i * MW : (i + 1) * MW], ot_sb[:])


def _build(hpc=HPC, n=N):
    nc = bacc.Bacc(
        "TRN2", target_bir_lowering=False, debug=False, num_devices=NCORES
    )
    qt = nc.dram_tensor("qt", [hpc, P, n], F32R, kind="ExternalInput").ap()
    kt = nc.dram_tensor("kt", [hpc, P, n], F32R, kind="ExternalInput").ap()
    v = nc.dram_tensor("v", [hpc, n, P], F32R, kind="ExternalInput").ap()
    mask = nc.dram_tensor("mask", [P, G, MW], F32R, kind="ExternalInput").ap()
    out = nc.dram_tensor("out", [hpc, P, n], F32, kind="ExternalOutput").ap()
    with tile.TileContext(nc) as tc:
        _emit_body(tc, qt, kt, v, mask, out, hpc, n)
    nc.compile()
    return nc


_NC_CACHE = {}


def _get_nc():
    if "nc" not in _NC_CACHE:
        _NC_CACHE["nc"] = _build()
    return _NC_CACHE["nc"]


def _make_mask():
    # mask[p, t, m] = 1 where query m (within the 512 super-tile) >= key t*128+p
    m = np.arange(MW)[None, None, :]
    t = np.arange(G)[None, :, None]
    p = np.arange(P)[:, None, None]
    return (m >= t * P + p).astype(np.float32)


def _prep(q, k, v):
    """Host-side reshaping/folding. Returns per-core in_maps and eq for post."""
    q = np.asarray(q, dtype=np.float32).reshape(B * H, N, D)
    k = np.asarray(k, dtype=np.float32).reshape(B * H, N, D)
    v = np.asarray(v, dtype=np.float32).reshape(B * H, N, D)

    qT = np.ascontiguousarray(q.transpose(0, 2, 1)) * np.float32(2.0 * SM)
    kT = np.ascontiguousarray(k.transpose(0, 2, 1))
    ek = np.exp(np.float32(-SM) * np.einsum("hnd,hnd->hn", k, k)).astype(np.float32)
    eq = np.exp(np.float32(-SM) * np.einsum("hnd,hnd->hn", q, q)).astype(np.float32)
    vs = (v * ek[:, :, None]).astype(np.float32)

    mask = _make_mask()
    in_maps = []
    for c in range(NCORES):
        s = slice(c * HPC, (c + 1) * HPC)
        in_maps.append(
            {
                "qt": np.ascontiguousarray(qT[s]),
                "kt": np.ascontiguousarray(kT[s]),
                "v": np.ascontiguousarray(vs[s]),
                "mask": mask,
            }
        )
    return in_maps, eq


def _run(in_maps, trace=False):
    nc = _get_nc()
    res = bass_utils.run_bass_kernel_spmd(
        nc, in_maps, core_ids=list(range(NCORES)), trace=trace
    )
    return res


def _post(res_list, eq):
    # res_list: per-core dicts with "out" [HPC, 128(d), N(m)]
    ot = np.concatenate([r["out"] for r in res_list], axis=0)  # [B*H, D, N]
    o = ot.transpose(0, 2, 1) * eq[:, :, None]  # [B*H, N, D]
    return np.ascontiguousarray(o.reshape(B, H, N, D).astype(np.float32))


def kernel(q, k, v):
    in_maps, eq = _prep(q, k, v)
    res = _run(in_maps, trace=False)
    return _post(res.results, eq)


# revision 4
# speedup vs baseline: 1.6862x; 1.6862x over previous
"""Causal RBF (non-softmax) attention on 8 Trainium2 NeuronCores.

Problem: q,k,v [B=2, H=16, N=2048, D=128] f32.
  logits = 2s*q@k^T - s*||q||^2 - s*||k||^2   (s = 1/sqrt(D))
  p = exp(logits) with causal mask; out = p @ v      (no softmax normalization)

Sharding: B*H = 32 heads -> 4 heads per core, fully independent.

Algebra used to make the device kernel cheap:
  out[m,:] = eq[m] * sum_{n<=m} exp(2s*q_m.k_n) * (ek[n]*v[n,:])
  with eq[m] = exp(-s*||q_m||^2), ek[n] = exp(-s*||k_n||^2).
The host folds 2s into qT, ek into v, and applies eq to the output rows, so
the device computes only:  ST = KT^T.T @ QT blocks -> Exp -> mask -> @ V'.

Device layouts (per head):
  qT [128(d), 2048(m)]  (host-transposed, scaled by 2s)
  kT [128(d), 2048(n)]  (host-transposed)
  v' [2048(n), 128(d)]  (ek-scaled, natural)
Output is written transposed, OT [128(d), 2048(m)]; host transposes back.

Matmuls run as float32r (full PE rate at free-dim 512). PSUM: ST group tile
[128, 4, 512] double-buffered = all 8 banks; PV matmuls accumulate into bank 0
of their own group's tile (legal: exp already consumed it), and DVE adds the
per-group partial into an SBUF accumulator.
"""

import math
import os
import sys

import numpy as np

sys.path.insert(0, "/opt/trn_rl_repo")

import concourse.bass as bass
import concourse.mybir as mybir
import concourse.tile as tile
from concourse import bacc, bass_utils

F32 = mybir.dt.float32
F32R = mybir.dt.float32r
EXP = mybir.ActivationFunctionType.Exp

B, H, N, D = 2, 16, 2048, 128
SM = 1.0 / math.sqrt(D)
P = 128
NCORES = 8
HPC = (B * H) // NCORES  # heads per core
MW = 512                 # m (query) super-tile width
G = 4                    # k-blocks per group (4 x 512 f32 = 4 PSUM banks)


def _emit_body(tc, qt, kt, v, cmask, out, hpc, n):
    """Per super-tile i: PE accumulates all PV matmuls straight into a PSUM
    OT tile; ST groups of 3 k-blocks (3 banks x 2 bufs) feed one Exp each.
    Diagonal 4-block group is packed into 3 banks with narrowed matmuls:
      bank0 = t0 [m 0:512), bank1 = t2 [m 256:512) | t3 [m 256:512),
      bank2 = t1 [m 128:512) | 128 pad (pad never computed or exp'd:
      exp covers flat cols [0,1408)).
    Causal masking = tiny DVE multiplies on the 128-col triangles only.
    """
    nc = tc.nc
    nb = n // P     # 128-wide k blocks
    mi = n // MW    # query super tiles
    from contextlib import ExitStack

    with ExitStack() as ctx:
        const = ctx.enter_context(tc.tile_pool(name="const", bufs=1))
        qk_pool = ctx.enter_context(tc.tile_pool(name="qk", bufs=2))
        v_pool = ctx.enter_context(tc.tile_pool(name="vp", bufs=2))
        st_pool = ctx.enter_context(tc.tile_pool(name="st", bufs=2, space="PSUM"))
        otp_pool = ctx.enter_context(tc.tile_pool(name="otp", bufs=2, space="PSUM"))
        pt_pool = ctx.enter_context(tc.tile_pool(name="pt", bufs=3))
        osb_pool = ctx.enter_context(tc.tile_pool(name="osb", bufs=2))

        # cmask [P, 256] = [zeros(128) | upper-tri(128)]; tri = cols 128:256
        cm_sb = const.tile([P, 2 * P], F32R)
        nc.sync.dma_start(cm_sb[:], cmask[:])
        ztri = cm_sb[:, 0 : 2 * P]
        tri = cm_sb[:, P : 2 * P]

        for h in range(hpc):
            qt_sb = qk_pool.tile([P, n], F32R, tag="qt")
            kt_sb = qk_pool.tile([P, n], F32R, tag="kt")
            v_sb = v_pool.tile([P, nb, P], F32R, tag="v")
            nc.sync.dma_start(qt_sb[:], qt[h])
            nc.sync.dma_start(kt_sb[:], kt[h])
            nc.sync.dma_start(v_sb[:], v[h].rearrange("(nb p) d -> p nb d", p=P))

            for i in range(mi):
                qs = qt_sb[:, i * MW : (i + 1) * MW]
                ot = otp_pool.tile([P, MW], F32, tag="otp")
                first_pv = [True]

                def pv(j, rhs, osl, stop=False):
                    nc.tensor.matmul(
                        osl, lhsT=v_sb[:, j, :], rhs=rhs,
                        start=first_pv[0], stop=stop,
                    )
                    first_pv[0] = False

                # ---- full groups: k-blocks [0, 4i) in chunks of 3 ----
                fullb = list(range(4 * i))
                for c0 in range(0, len(fullb), 3):
                    chunk = fullb[c0 : c0 + 3]
                    glen = len(chunk)
                    st = st_pool.tile([P, 3, MW], F32, tag="st")
                    pt = pt_pool.tile([P, 3, MW], F32R, tag="pt")
                    for idx, j in enumerate(chunk):
                        nc.tensor.matmul(
                            st[:, idx, :], lhsT=kt_sb[:, j * P : (j + 1) * P],
                            rhs=qs, start=True, stop=True,
                        )
                    nc.scalar.activation(
                        pt[:, :glen, :], st[:, :glen, :], EXP
                    )
                    for idx, j in enumerate(chunk):
                        pv(j, pt[:, idx, :], ot[:, :])

                # ---- diagonal group: k-blocks [4i, 4i+4), packed/narrowed ----
                st = st_pool.tile([P, 3, MW], F32, tag="st")
                pt = pt_pool.tile([P, 3, MW], F32R, tag="pt")
                jb = 4 * i
                # t0 full width
                nc.tensor.matmul(st[:, 0, :], lhsT=kt_sb[:, jb * P : (jb + 1) * P],
                                 rhs=qs, start=True, stop=True)
                # t2: m [256:512) -> bank1 cols 0:256
                nc.tensor.matmul(st[:, 1, 0:256],
                                 lhsT=kt_sb[:, (jb + 2) * P : (jb + 3) * P],
                                 rhs=qs[:, 256:512], start=True, stop=True)
                # t3: m [256:512) -> bank1 cols 256:512
                nc.tensor.matmul(st[:, 1, 256:512],
                                 lhsT=kt_sb[:, (jb + 3) * P : (jb + 4) * P],
                                 rhs=qs[:, 256:512], start=True, stop=True)
                # t1: m [128:512) -> bank2 cols 0:384
                nc.tensor.matmul(st[:, 2, 0:384],
                                 lhsT=kt_sb[:, (jb + 1) * P : (jb + 2) * P],
                                 rhs=qs[:, 128:512], start=True, stop=True)
                st_flat = st.rearrange("p a b -> p (a b)")
                pt_flat = pt.rearrange("p a b -> p (a b)")
                nc.scalar.activation(pt_flat[:, 0:1408], st_flat[:, 0:1408], EXP)
                # triangle masks (and zero block for t3)
                nc.vector.tensor_mul(pt[:, 0, 0:P], pt[:, 0, 0:P], tri)
                nc.vector.tensor_mul(pt[:, 2, 0:P], pt[:, 2, 0:P], tri)
                nc.vector.tensor_mul(pt[:, 1, 0:P], pt[:, 1, 0:P], tri)
                nc.vector.tensor_mul(pt[:, 1, 256:512], pt[:, 1, 256:512], ztri)
                pv(jb + 0, pt[:, 0, :], ot[:, :])
                pv(jb + 1, pt[:, 2, 0:384], ot[:, 128:512])
                pv(jb + 2, pt[:, 1, 0:256], ot[:, 256:512])
                pv(jb + 3, pt[:, 1, 256:512], ot[:, 256:512], stop=True)

                out_sb = osb_pool.tile([P, MW], F32, tag="osb")
                nc.vector.tensor_copy(out_sb[:], ot[:])
                nc.sync.dma_start(out[h, :, i * MW : (i + 1) * MW], out_sb[:])


def _build(hpc=HPC, n=N):
    nc = bacc.Bacc(
        "TRN2", target_bir_lowering=False, debug=False, num_devices=NCORES
    )
    qt = nc.dram_tensor("qt", [hpc, P, n], F32R, kind="ExternalInput").ap()
    kt = nc.dram_tensor("kt", [hpc, P, n], F32R, kind="ExternalInput").ap()
    v = nc.dram_tensor("v", [hpc, n, P], F32R, kind="ExternalInput").ap()
    cmask = nc.dram_tensor("cmask", [P, 2 * P], F32R, kind="ExternalInput").ap()
    out = nc.dram_tensor("out", [hpc, P, n], F32, kind="ExternalOutput").ap()
    with tile.TileContext(nc) as tc:
        _emit_body(tc, qt, kt, v, cmask, out, hpc, n)
    nc.compile()
    return nc


_NC_CACHE = {}


def _get_nc():
    if "nc" not in _NC_CACHE:
        _NC_CACHE["nc"] = _build()
    return _NC_CACHE["nc"]


def _make_mask():
    # cmask [P, 256] = [zeros(128) | tri(128)], tri[p, c] = 1 where c >= p
    z = np.zeros((P, P), dtype=np.float32)
    c = np.arange(P)[None, :]
    p = np.arange(P)[:, None]
    tri = (c >= p).astype(np.float32)
    return np.concatenate([z, tri], axis=1)


def _prep(q, k, v):
    """Host-side reshaping/folding. Returns per-core in_maps and eq for post."""
    q = np.asarray(q, dtype=np.float32).reshape(B * H, N, D)
    k = np.asarray(k, dtype=np.float32).reshape(B * H, N, D)
    v = np.asarray(v, dtype=np.float32).reshape(B * H, N, D)

    qT = np.ascontiguousarray(q.transpose(0, 2, 1)) * np.float32(2.0 * SM)
    kT = np.ascontiguousarray(k.transpose(0, 2, 1))
    ek = np.exp(np.float32(-SM) * np.einsum("hnd,hnd->hn", k, k)).astype(np.float32)
    eq = np.exp(np.float32(-SM) * np.einsum("hnd,hnd->hn", q, q)).astype(np.float32)
    vs = (v * ek[:, :, None]).astype(np.float32)

    mask = _make_mask()
    in_maps = []
    for c in range(NCORES):
        s = slice(c * HPC, (c + 1) * HPC)
        in_maps.append(
            {
                "qt": np.ascontiguousarray(qT[s]),
                "kt": np.ascontiguousarray(kT[s]),
                "v": np.ascontiguousarray(vs[s]),
                "cmask": mask,
            }
        )
    return in_maps, eq


def _run(in_maps, trace=False):
    nc = _get_nc()
    res = bass_utils.run_bass_kernel_spmd(
        nc, in_maps, core_ids=list(range(NCORES)), trace=trace
    )
    return res


def _post(res_list, eq):
    # res_list: per-core dicts with "out" [HPC, 128(d), N(m)]
    ot = np.concatenate([r["out"] for r in res_list], axis=0)  # [B*H, D, N]
    o = ot.transpose(0, 2, 1) * eq[:, :, None]  # [B*H, N, D]
    return np.ascontiguousarray(o.reshape(B, H, N, D).astype(np.float32))


def kernel(q, k, v):
    in_maps, eq = _prep(q, k, v)
    res = _run(in_maps, trace=False)
    return _post(res.results, eq)


# revision 6
# speedup vs baseline: 1.7036x; 1.0103x over previous
"""Causal RBF (non-softmax) attention on 8 Trainium2 NeuronCores.

Problem: q,k,v [B=2, H=16, N=2048, D=128] f32.
  logits = 2s*q@k^T - s*||q||^2 - s*||k||^2   (s = 1/sqrt(D))
  p = exp(logits) with causal mask; out = p @ v      (no softmax normalization)

Sharding: B*H = 32 heads -> 4 heads per core, fully independent.

Algebra used to make the device kernel cheap:
  out[m,:] = eq[m] * sum_{n<=m} exp(2s*q_m.k_n) * (ek[n]*v[n,:])
  with eq[m] = exp(-s*||q_m||^2), ek[n] = exp(-s*||k_n||^2).
The host folds 2s into qT, ek into v, and applies eq to the output rows, so
the device computes only:  ST = KT^T.T @ QT blocks -> Exp -> mask -> @ V'.

Device layouts (per head):
  qT [128(d), 2048(m)]  (host-transposed, scaled by 2s)
  kT [128(d), 2048(n)]  (host-transposed)
  v' [2048(n), 128(d)]  (ek-scaled, natural)
Output is written transposed, OT [128(d), 2048(m)]; host transposes back.

Matmuls run as float32r (full PE rate at free-dim 512). PSUM: ST group tile
[128, 4, 512] double-buffered = all 8 banks; PV matmuls accumulate into bank 0
of their own group's tile (legal: exp already consumed it), and DVE adds the
per-group partial into an SBUF accumulator.
"""

import math
import os
import sys

import numpy as np

sys.path.insert(0, "/opt/trn_rl_repo")

import concourse.bass as bass
import concourse.mybir as mybir
import concourse.tile as tile
from concourse import bacc, bass_utils

F32 = mybir.dt.float32
F32R = mybir.dt.float32r
EXP = mybir.ActivationFunctionType.Exp

B, H, N, D = 2, 16, 2048, 128
SM = 1.0 / math.sqrt(D)
P = 128
NCORES = 8
HPC = (B * H) // NCORES  # heads per core
MW = 512                 # m (query) super-tile width
G = 4                    # k-blocks per group (4 x 512 f32 = 4 PSUM banks)


def _emit_body(tc, qt, kt, v, cmask, out, hpc, n):
    """Per super-tile i: PE accumulates all PV matmuls straight into a PSUM
    OT tile; ST groups of 3 k-blocks (3 banks x 2 bufs) feed one Exp each.
    Diagonal 4-block group is packed into 3 banks with narrowed matmuls:
      bank0 = t0 [m 0:512), bank1 = t2 [m 256:512) | t3 [m 256:512),
      bank2 = t1 [m 128:512) | 128 pad (pad never computed or exp'd:
      exp covers flat cols [0,1408)).
    Causal masking = tiny DVE multiplies on the 128-col triangles only.
    """
    nc = tc.nc
    nb = n // P     # 128-wide k blocks
    mi = n // MW    # query super tiles
    from contextlib import ExitStack

    with ExitStack() as ctx:
        const = ctx.enter_context(tc.tile_pool(name="const", bufs=1))
        qk_pool = ctx.enter_context(tc.tile_pool(name="qk", bufs=2))
        v_pool = ctx.enter_context(tc.tile_pool(name="vp", bufs=2))
        st_pool = ctx.enter_context(tc.tile_pool(name="st", bufs=2, space="PSUM"))
        otp_pool = ctx.enter_context(tc.tile_pool(name="otp", bufs=2, space="PSUM"))
        pt_pool = ctx.enter_context(tc.tile_pool(name="pt", bufs=4))
        osb_pool = ctx.enter_context(tc.tile_pool(name="osb", bufs=2))

        # cmask [P, 256] = [zeros(128) | upper-tri(128)]; tri = cols 128:256
        cm_sb = const.tile([P, 2 * P], F32R)
        nc.sync.dma_start(cm_sb[:], cmask[:])
        ztri = cm_sb[:, 0 : 2 * P]
        tri = cm_sb[:, P : 2 * P]

        for h in range(hpc):
            # chunked loads (quarter-head each) so compute starts after the
            # first ~0.75 MiB instead of the full 3 MiB head
            qt_c, kt_c, v_c = [], [], []
            for c in range(mi):
                qtc = qk_pool.tile([P, MW], F32R, tag=f"qt{c}")
                ktc = qk_pool.tile([P, MW], F32R, tag=f"kt{c}")
                vc = v_pool.tile([P, G, P], F32R, tag=f"v{c}")
                nc.sync.dma_start(qtc[:], qt[h, :, c * MW : (c + 1) * MW])
                nc.sync.dma_start(ktc[:], kt[h, :, c * MW : (c + 1) * MW])
                nc.sync.dma_start(
                    vc[:],
                    v[h, c * G * P : (c + 1) * G * P].rearrange(
                        "(nb p) d -> p nb d", p=P
                    ),
                )
                qt_c.append(qtc)
                kt_c.append(ktc)
                v_c.append(vc)

            def kt_blk(j):
                return kt_c[j // G][:, (j % G) * P : (j % G + 1) * P]

            def v_blk(j):
                return v_c[j // G][:, j % G, :]

            for i in range(mi):
                qs = qt_c[i][:]
                ot = otp_pool.tile([P, MW], F32, tag="otp")
                first_pv = [True]

                def pv(j, rhs, osl, stop=False):
                    nc.tensor.matmul(
                        osl, lhsT=v_blk(j), rhs=rhs,
                        start=first_pv[0], stop=stop,
                    )
                    first_pv[0] = False

                # ---- full groups: k-blocks [0, 4i) in chunks of 3 ----
                fullb = list(range(4 * i))
                for c0 in range(0, len(fullb), 3):
                    chunk = fullb[c0 : c0 + 3]
                    glen = len(chunk)
                    st = st_pool.tile([P, 3, MW], F32, tag="st")
                    pt = pt_pool.tile([P, 3, MW], F32R, tag="pt")
                    for idx, j in enumerate(chunk):
                        nc.tensor.matmul(
                            st[:, idx, :], lhsT=kt_blk(j),
                            rhs=qs, start=True, stop=True,
                        )
                    nc.scalar.activation(
                        pt[:, :glen, :], st[:, :glen, :], EXP
                    )
                    for idx, j in enumerate(chunk):
                        pv(j, pt[:, idx, :], ot[:, :])

                # ---- diagonal group: k-blocks [4i, 4i+4), packed/narrowed ----
                st = st_pool.tile([P, 3, MW], F32, tag="st")
                pt = pt_pool.tile([P, 3, MW], F32R, tag="pt")
                jb = 4 * i
                # t0 full width
                nc.tensor.matmul(st[:, 0, :], lhsT=kt_blk(jb),
                                 rhs=qs, start=True, stop=True)
                # t2: m [256:512) -> bank1 cols 0:256
                nc.tensor.matmul(st[:, 1, 0:256],
                                 lhsT=kt_blk(jb + 2),
                                 rhs=qs[:, 256:512], start=True, stop=True)
                # t3: m [256:512) -> bank1 cols 256:512
                nc.tensor.matmul(st[:, 1, 256:512],
                                 lhsT=kt_blk(jb + 3),
                                 rhs=qs[:, 256:512], start=True, stop=True)
                # t1: m [128:512) -> bank2 cols 0:384
                nc.tensor.matmul(st[:, 2, 0:384],
                                 lhsT=kt_blk(jb + 1),
                                 rhs=qs[:, 128:512], start=True, stop=True)
                st_flat = st.rearrange("p a b -> p (a b)")
                pt_flat = pt.rearrange("p a b -> p (a b)")
                nc.scalar.activation(pt_flat[:, 0:1408], st_flat[:, 0:1408], EXP)
                # triangle masks (and zero block for t3)
                nc.vector.tensor_mul(pt[:, 0, 0:P], pt[:, 0, 0:P], tri)
                nc.vector.tensor_mul(pt[:, 2, 0:P], pt[:, 2, 0:P], tri)
                nc.vector.tensor_mul(pt[:, 1, 0:P], pt[:, 1, 0:P], tri)
                nc.vector.tensor_mul(pt[:, 1, 256:512], pt[:, 1, 256:512], ztri)
                pv(jb + 0, pt[:, 0, :], ot[:, :])
                pv(jb + 1, pt[:, 2, 0:384], ot[:, 128:512])
                pv(jb + 2, pt[:, 1, 0:256], ot[:, 256:512])
                pv(jb + 3, pt[:, 1, 256:512], ot[:, 256:512], stop=True)

                out_sb = osb_pool.tile([P, MW], F32, tag="osb")
                nc.vector.tensor_copy(out_sb[:], ot[:])
                nc.sync.dma_start(out[h, :, i * MW : (i + 1) * MW], out_sb[:])


def _build(hpc=HPC, n=N):
    nc = bacc.Bacc(
        "TRN2", target_bir_lowering=False, debug=False, num_devices=NCORES
    )
    qt = nc.dram_tensor("qt", [hpc, P, n], F32R, kind="ExternalInput").ap()
    kt = nc.dram_tensor("kt", [hpc, P, n], F32R, kind="ExternalInput").ap()
    v = nc.dram_tensor("v", [hpc, n, P], F32R, kind="ExternalInput").ap()
    cmask = nc.dram_tensor("cmask", [P, 2 * P], F32R, kind="ExternalInput").ap()
    out = nc.dram_tensor("out", [hpc, P, n], F32, kind="ExternalOutput").ap()
    with tile.TileContext(nc) as tc:
        _emit_body(tc, qt, kt, v, cmask, out, hpc, n)
    nc.compile()
    return nc


_NC_CACHE = {}


def _get_nc():
    if "nc" not in _NC_CACHE:
        _NC_CACHE["nc"] = _build()
    return _NC_CACHE["nc"]


def _make_mask():
    # cmask [P, 256] = [zeros(128) | tri(128)], tri[p, c] = 1 where c >= p
    z = np.zeros((P, P), dtype=np.float32)
    c = np.arange(P)[None, :]
    p = np.arange(P)[:, None]
    tri = (c >= p).astype(np.float32)
    return np.concatenate([z, tri], axis=1)


def _prep(q, k, v):
    """Host-side reshaping/folding. Returns per-core in_maps and eq for post."""
    q = np.asarray(q, dtype=np.float32).reshape(B * H, N, D)
    k = np.asarray(k, dtype=np.float32).reshape(B * H, N, D)
    v = np.asarray(v, dtype=np.float32).reshape(B * H, N, D)

    qT = np.ascontiguousarray(q.transpose(0, 2, 1)) * np.float32(2.0 * SM)
    kT = np.ascontiguousarray(k.transpose(0, 2, 1))
    ek = np.exp(np.float32(-SM) * np.einsum("hnd,hnd->hn", k, k)).astype(np.float32)
    eq = np.exp(np.float32(-SM) * np.einsum("hnd,hnd->hn", q, q)).astype(np.float32)
    vs = (v * ek[:, :, None]).astype(np.float32)

    mask = _make_mask()
    in_maps = []
    for c in range(NCORES):
        s = slice(c * HPC, (c + 1) * HPC)
        in_maps.append(
            {
                "qt": np.ascontiguousarray(qT[s]),
                "kt": np.ascontiguousarray(kT[s]),
                "v": np.ascontiguousarray(vs[s]),
                "cmask": mask,
            }
        )
    return in_maps, eq


def _run(in_maps, trace=False):
    nc = _get_nc()
    res = bass_utils.run_bass_kernel_spmd(
        nc, in_maps, core_ids=list(range(NCORES)), trace=trace
    )
    return res


def _post(res_list, eq):
    # res_list: per-core dicts with "out" [HPC, 128(d), N(m)]
    ot = np.concatenate([r["out"] for r in res_list], axis=0)  # [B*H, D, N]
    o = ot.transpose(0, 2, 1) * eq[:, :, None]  # [B*H, N, D]
    return np.ascontiguousarray(o.reshape(B, H, N, D).astype(np.float32))


def kernel(q, k, v):
    in_maps, eq = _prep(q, k, v)
    res = _run(in_maps, trace=False)
    return _post(res.results, eq)


# revision 12
# speedup vs baseline: 1.7716x; 1.0399x over previous
"""Causal RBF (non-softmax) attention on 8 Trainium2 NeuronCores.

Problem: q,k,v [B=2, H=16, N=2048, D=128] f32.
  logits = 2s*q@k^T - s*||q||^2 - s*||k||^2   (s = 1/sqrt(D))
  p = exp(logits) with causal mask; out = p @ v      (no softmax normalization)

Sharding: B*H = 32 heads -> 4 heads per core, fully independent.

Algebra used to make the device kernel cheap:
  out[m,:] = eq[m] * sum_{n<=m} exp(2s*q_m.k_n) * (ek[n]*v[n,:])
  with eq[m] = exp(-s*||q_m||^2), ek[n] = exp(-s*||k_n||^2).
The host folds 2s into qT, ek into v, and applies eq to the output rows, so
the device computes only:  ST = KT^T.T @ QT blocks -> Exp -> mask -> @ V'.

Device layouts (per head):
  qT [128(d), 2048(m)]  (host-transposed, scaled by 2s)
  kT [128(d), 2048(n)]  (host-transposed)
  v' [2048(n), 128(d)]  (ek-scaled, natural)
Output is written transposed, OT [128(d), 2048(m)]; host transposes back.

Matmuls run as float32r (full PE rate at free-dim 512). PSUM: ST group tile
[128, 4, 512] double-buffered = all 8 banks; PV matmuls accumulate into bank 0
of their own group's tile (legal: exp already consumed it), and DVE adds the
per-group partial into an SBUF accumulator.
"""

import math
import os
import sys
import time

import numpy as np

sys.path.insert(0, "/opt/trn_rl_repo")

import concourse.bass as bass
import concourse.mybir as mybir
import concourse.tile as tile
from concourse import bacc, bass_utils

F32 = mybir.dt.float32
F32R = mybir.dt.float32r
EXP = mybir.ActivationFunctionType.Exp

B, H, N, D = 2, 16, 2048, 128
SM = 1.0 / math.sqrt(D)
P = 128
NCORES = 8
HPC = (B * H) // NCORES  # heads per core
MW = 512                 # m (query) super-tile width
G = 4                    # k-blocks per group (4 x 512 f32 = 4 PSUM banks)


def _emit_body(tc, qt, kt, v, cmask, out, hpc, n):
    """Software-pipelined emission: for the flat list of (supertile, group)
    work items, group k+1's ST matmuls + Exp are emitted BEFORE group k's
    masks/PV matmuls, so the scheduler always has PE work queued while ACT
    or DVE finish the previous group. Head h+1's chunked loads are emitted
    at the start of head h (a full head of DMA lead time).

    Per group: 3 ST matmuls (f32r, 3 PSUM banks) -> one Exp -> PV matmuls
    accumulating straight into a per-supertile PSUM OT tile. The diagonal
    4-block group packs narrowed matmuls into 3 banks:
      bank0 = t0 [m 0:512); bank1 = t2 | t3 (both [m 256:512));
      bank2 = t1 [m 128:512) | 128 unused cols (exp covers flat [0,1408)).
    Causal masking = tiny DVE multiplies on 128-col triangles only.
    """
    nc = tc.nc
    mi = n // MW    # query super tiles per head
    from contextlib import ExitStack

    with ExitStack() as ctx:
        const = ctx.enter_context(tc.tile_pool(name="const", bufs=1))
        qk_pool = ctx.enter_context(tc.tile_pool(name="qk", bufs=3))
        v_pool = ctx.enter_context(tc.tile_pool(name="vp", bufs=3))
        st_pool = ctx.enter_context(tc.tile_pool(name="st", bufs=2, space="PSUM"))
        otp_pool = ctx.enter_context(tc.tile_pool(name="otp", bufs=2, space="PSUM"))
        pt_pool = ctx.enter_context(tc.tile_pool(name="pt", bufs=4))
        osb_pool = ctx.enter_context(tc.tile_pool(name="osb", bufs=2))

        # cmask [P, 256] = [zeros(128) | upper-tri(128)]; tri = cols 128:256
        cm_sb = const.tile([P, 2 * P], F32R)
        nc.sync.dma_start(cm_sb[:], cmask[:])
        ztri = cm_sb[:, 0 : 2 * P]
        tri = cm_sb[:, P : 2 * P]

        head_tiles = {}

        def emit_loads(h):
            qt_c, kt_c, v_c = [], [], []
            for c in range(mi):
                qtc = qk_pool.tile([P, MW], F32R, tag=f"qt{c}")
                ktc = qk_pool.tile([P, MW], F32R, tag=f"kt{c}")
                vc = v_pool.tile([P, G, P], F32R, tag=f"v{c}")
                nc.sync.dma_start(qtc[:], qt[h, :, c * MW : (c + 1) * MW])
                nc.sync.dma_start(ktc[:], kt[h, :, c * MW : (c + 1) * MW])
                nc.sync.dma_start(
                    vc[:],
                    v[h, c * G * P : (c + 1) * G * P].rearrange(
                        "(nb p) d -> p nb d", p=P
                    ),
                )
                qt_c.append(qtc)
                kt_c.append(ktc)
                v_c.append(vc)
            head_tiles[h] = (qt_c, kt_c, v_c)

        # flat work list: (h, i, chunk-of-k-blocks-or-"diag", is_last_group)
        work = []
        for h in range(hpc):
            for i in range(mi):
                fullb = list(range(4 * i))
                for c0 in range(0, len(fullb), 3):
                    work.append((h, i, fullb[c0 : c0 + 3], False))
                work.append((h, i, None, True))  # diag group

        ustate = {}  # (h,i) -> dict(ot=..., first=...)
        pend = {}    # k -> (st, pt) tiles

        def kt_blk(h, j):
            return head_tiles[h][1][j // G][:, (j % G) * P : (j % G + 1) * P]

        def v_blk(h, j):
            return head_tiles[h][2][j // G][:, j % G, :]

        def st_exp(k):
            h, i, chunk, isdiag_last = work[k]
            if i == 0 and (chunk is None or chunk == []) and h + 1 < hpc:
                # first group of head h: prefetch head h+1's tensors
                emit_loads(h + 1)
            qs = head_tiles[h][0][i][:]
            st = st_pool.tile([P, 3, MW], F32, tag="st")
            pt = pt_pool.tile([P, 3, MW], F32R, tag="pt")
            if chunk is not None:
                for idx, j in enumerate(chunk):
                    nc.tensor.matmul(
                        st[:, idx, :], lhsT=kt_blk(h, j),
                        rhs=qs, start=True, stop=True,
                    )
                nc.scalar.activation(
                    pt[:, : len(chunk), :], st[:, : len(chunk), :], EXP
                )
            else:
                jb = 4 * i
                nc.tensor.matmul(st[:, 0, :], lhsT=kt_blk(h, jb),
                                 rhs=qs, start=True, stop=True)
                nc.tensor.matmul(st[:, 1, 0:256], lhsT=kt_blk(h, jb + 2),
                                 rhs=qs[:, 256:512], start=True, stop=True)
                nc.tensor.matmul(st[:, 1, 256:512], lhsT=kt_blk(h, jb + 3),
                                 rhs=qs[:, 256:512], start=True, stop=True)
                nc.tensor.matmul(st[:, 2, 0:384], lhsT=kt_blk(h, jb + 1),
                                 rhs=qs[:, 128:512], start=True, stop=True)
                st_flat = st.rearrange("p a b -> p (a b)")
                pt_flat = pt.rearrange("p a b -> p (a b)")
                nc.scalar.activation(pt_flat[:, 0:1408], st_flat[:, 0:1408], EXP)
            pend[k] = (st, pt)

        def finish(k):
            h, i, chunk, islast = work[k]
            st, pt = pend.pop(k)
            u = ustate.get((h, i))
            if u is None:
                ot_tile = otp_pool.tile([P, MW], F32, tag="otp", name="ot_tile")
                u = ustate[(h, i)] = {"ot": ot_tile, "first": True}
            ot = u["ot"]

            def pv(j, rhs, osl, stop=False):
                nc.tensor.matmul(osl, lhsT=v_blk(h, j), rhs=rhs,
                                 start=u["first"], stop=stop)
                u["first"] = False

            if chunk is not None:
                for idx, j in enumerate(chunk):
                    pv(j, pt[:, idx, :], ot[:, :])
            else:
                jb = 4 * i
                nc.vector.tensor_mul(pt[:, 0, 0:P], pt[:, 0, 0:P], tri)
                nc.vector.tensor_mul(pt[:, 2, 0:P], pt[:, 2, 0:P], tri)
                nc.vector.tensor_mul(pt[:, 1, 0:P], pt[:, 1, 0:P], tri)
                nc.vector.tensor_mul(pt[:, 1, 256:512], pt[:, 1, 256:512], ztri)
                pv(jb + 0, pt[:, 0, :], ot[:, :])
                pv(jb + 1, pt[:, 2, 0:384], ot[:, 128:512])
                pv(jb + 2, pt[:, 1, 0:256], ot[:, 256:512])
                pv(jb + 3, pt[:, 1, 256:512], ot[:, 256:512], stop=True)
                # close out the supertile
                out_sb = osb_pool.tile([P, MW], F32, tag="osb")
                nc.vector.tensor_copy(out_sb[:], ot[:])
                nc.gpsimd.dma_start(out[h, :, i * MW : (i + 1) * MW], out_sb[:])

        emit_loads(0)
        st_exp(0)
        for k in range(len(work)):
            if k + 1 < len(work):
                st_exp(k + 1)
            finish(k)


def _build(hpc=HPC, n=N):
    nc = bacc.Bacc(
        "TRN2", target_bir_lowering=False, debug=False, num_devices=NCORES
    )
    qt = nc.dram_tensor("qt", [hpc, P, n], F32R, kind="ExternalInput").ap()
    kt = nc.dram_tensor("kt", [hpc, P, n], F32R, kind="ExternalInput").ap()
    v = nc.dram_tensor("v", [hpc, n, P], F32R, kind="ExternalInput").ap()
    cmask = nc.dram_tensor("cmask", [P, 2 * P], F32R, kind="ExternalInput").ap()
    out = nc.dram_tensor("out", [hpc, P, n], F32, kind="ExternalOutput").ap()
    with tile.TileContext(nc) as tc:
        _emit_body(tc, qt, kt, v, cmask, out, hpc, n)
    nc.compile()
    return nc


_NC_CACHE = {}


def _get_nc():
    if "nc" not in _NC_CACHE:
        _NC_CACHE["nc"] = _build()
    return _NC_CACHE["nc"]


def _make_mask():
    # cmask [P, 256] = [zeros(128) | tri(128)], tri[p, c] = 1 where c >= p
    z = np.zeros((P, P), dtype=np.float32)
    c = np.arange(P)[None, :]
    p = np.arange(P)[:, None]
    tri = (c >= p).astype(np.float32)
    return np.concatenate([z, tri], axis=1)


def _prep(q, k, v):
    """Host-side reshaping/folding. Returns per-core in_maps and eq for post."""
    q = np.asarray(q, dtype=np.float32).reshape(B * H, N, D)
    k = np.asarray(k, dtype=np.float32).reshape(B * H, N, D)
    v = np.asarray(v, dtype=np.float32).reshape(B * H, N, D)

    qT = np.ascontiguousarray(q.transpose(0, 2, 1)) * np.float32(2.0 * SM)
    kT = np.ascontiguousarray(k.transpose(0, 2, 1))
    ek = np.exp(np.float32(-SM) * np.einsum("hnd,hnd->hn", k, k)).astype(np.float32)
    eq = np.exp(np.float32(-SM) * np.einsum("hnd,hnd->hn", q, q)).astype(np.float32)
    vs = (v * ek[:, :, None]).astype(np.float32)

    mask = _make_mask()
    in_maps = []
    for c in range(NCORES):
        s = slice(c * HPC, (c + 1) * HPC)
        in_maps.append(
            {
                "qt": np.ascontiguousarray(qT[s]),
                "kt": np.ascontiguousarray(kT[s]),
                "v": np.ascontiguousarray(vs[s]),
                "cmask": mask,
            }
        )
    return in_maps, eq


def _run(in_maps, trace=False):
    nc = _get_nc()
    res = bass_utils.run_bass_kernel_spmd(
        nc, in_maps, core_ids=list(range(NCORES)), trace=trace
    )
    return res


def _post(res_list, eq):
    # res_list: per-core dicts with "out" [HPC, 128(d), N(m)]
    ot = np.concatenate([r["out"] for r in res_list], axis=0)  # [B*H, D, N]
    o = ot.transpose(0, 2, 1) * eq[:, :, None]  # [B*H, N, D]
    return np.ascontiguousarray(o.reshape(B, H, N, D).astype(np.float32))


def kernel(q, k, v):
    in_maps, eq = _prep(q, k, v)
    last_err = None
    for attempt in range(3):
        try:
            res = _run(in_maps, trace=False)
            return _post(res.results, eq)
        except Exception as e:  # axon/NRT first-run flakiness: retry
            last_err = e
            time.sleep(2.0)
    raise last_err


# revision 14
# speedup vs baseline: 1.9026x; 1.0739x over previous
"""Causal RBF (non-softmax) attention on 8 Trainium2 NeuronCores.

Problem: q,k,v [B=2, H=16, N=2048, D=128] f32.
  logits = 2s*q@k^T - s*||q||^2 - s*||k||^2   (s = 1/sqrt(D))
  p = exp(logits) with causal mask; out = p @ v      (no softmax normalization)

Sharding: B*H = 32 heads -> 4 heads per core, fully independent.

Algebra used to make the device kernel cheap:
  out[m,:] = eq[m] * sum_{n<=m} exp(2s*q_m.k_n) * (ek[n]*v[n,:])
  with eq[m] = exp(-s*||q_m||^2), ek[n] = exp(-s*||k_n||^2).
The host folds 2s into qT, ek into v, and applies eq to the output rows, so
the device computes only:  ST = KT^T.T @ QT blocks -> Exp -> mask -> @ V'.

Device layouts (per head):
  qT [128(d), 2048(m)]  (host-transposed, scaled by 2s)
  kT [128(d), 2048(n)]  (host-transposed)
  v' [2048(n), 128(d)]  (ek-scaled, natural)
Output is written transposed, OT [128(d), 2048(m)]; host transposes back.

Matmuls run as float32r (full PE rate at free-dim >= 256; ~3e-4 rel err from
its tf32-like rounding). PSUM: ST group tiles [128, 3, 512] double-buffered
(6 banks) + per-supertile OT accumulator tiles (2 banks); PV matmuls
accumulate straight into OT. Emission is software-pipelined one group ahead
so the PE always has queued work while ACT runs Exp. Measured on 8 axon trn2
cores: ~106 us NEFF exec, rel err 2.9e-4 (ACT/exp-roofline bound: 71 us of
pure EXP at 1 elem/lane/cycle is irreducible for causal N=2048 x 4 heads).
"""

import math
import sys
import time

import numpy as np

sys.path.insert(0, "/opt/trn_rl_repo")

import concourse.mybir as mybir
import concourse.tile as tile
from concourse import bacc, bass_utils

F32 = mybir.dt.float32
F32R = mybir.dt.float32r
EXP = mybir.ActivationFunctionType.Exp

B, H, N, D = 2, 16, 2048, 128
SM = 1.0 / math.sqrt(D)
P = 128
NCORES = 8
HPC = (B * H) // NCORES  # heads per core
MW = 512                 # m (query) super-tile width
G = 4                    # k-blocks per group (4 x 512 f32 = 4 PSUM banks)


def _emit_body(tc, qt, kt, v, cmask, out, hpc, n):
    """Software-pipelined emission: for the flat list of (supertile, group)
    work items, group k+1's ST matmuls + Exp are emitted BEFORE group k's
    masks/PV matmuls, so the scheduler always has PE work queued while ACT
    or DVE finish the previous group. Head h+1's chunked loads are emitted
    at the start of head h (a full head of DMA lead time).

    Per group: 3 ST matmuls (f32r, 3 PSUM banks) -> one Exp -> PV matmuls
    accumulating straight into a per-supertile PSUM OT tile. The diagonal
    4-block group packs narrowed matmuls into 3 banks:
      bank0 = t0 [m 0:512); bank1 = t2 | t3 (both [m 256:512));
      bank2 = t1 [m 128:512) | 128 unused cols (exp covers flat [0,1408)).
    Causal masking = tiny DVE multiplies on 128-col triangles only.
    """
    nc = tc.nc
    mi = n // MW    # query super tiles per head
    from contextlib import ExitStack

    with ExitStack() as ctx:
        const = ctx.enter_context(tc.tile_pool(name="const", bufs=1))
        qk_pool = ctx.enter_context(tc.tile_pool(name="qk", bufs=3))
        v_pool = ctx.enter_context(tc.tile_pool(name="vp", bufs=3))
        st_pool = ctx.enter_context(tc.tile_pool(name="st", bufs=2, space="PSUM"))
        otp_pool = ctx.enter_context(tc.tile_pool(name="otp", bufs=2, space="PSUM"))
        pt_pool = ctx.enter_context(tc.tile_pool(name="pt", bufs=4))
        osb_pool = ctx.enter_context(tc.tile_pool(name="osb", bufs=2))

        # cmask [P, 256] = [zeros(128) | upper-tri(128)]; tri = cols 128:256
        cm_sb = const.tile([P, 2 * P], F32R)
        nc.sync.dma_start(cm_sb[:], cmask[:])
        ztri = cm_sb[:, 0 : 2 * P]
        tri = cm_sb[:, P : 2 * P]

        head_tiles = {}

        def emit_loads(h):
            qt_c, kt_c, v_c = [], [], []
            for c in range(mi):
                qtc = qk_pool.tile([P, MW], F32R, tag=f"qt{c}")
                ktc = qk_pool.tile([P, MW], F32R, tag=f"kt{c}")
                vc = v_pool.tile([P, G, P], F32R, tag=f"v{c}")
                nc.sync.dma_start(qtc[:], qt[h, :, c * MW : (c + 1) * MW])
                nc.sync.dma_start(ktc[:], kt[h, :, c * MW : (c + 1) * MW])
                nc.sync.dma_start(
                    vc[:],
                    v[h, c * G * P : (c + 1) * G * P].rearrange(
                        "(nb p) d -> p nb d", p=P
                    ),
                )
                qt_c.append(qtc)
                kt_c.append(ktc)
                v_c.append(vc)
            head_tiles[h] = (qt_c, kt_c, v_c)

        # flat work list: (h, i, chunk-of-k-blocks-or-"diag", is_last_group)
        work = []
        for h in range(hpc):
            for i in range(mi):
                fullb = list(range(4 * i))
                for c0 in range(0, len(fullb), 3):
                    work.append((h, i, fullb[c0 : c0 + 3], False))
                work.append((h, i, None, True))  # diag group

        ustate = {}  # (h,i) -> dict(ot=..., first=...)
        pend = {}    # k -> (st, pt) tiles

        def kt_blk(h, j):
            return head_tiles[h][1][j // G][:, (j % G) * P : (j % G + 1) * P]

        def v_blk(h, j):
            return head_tiles[h][2][j // G][:, j % G, :]

        def st_exp(k):
            h, i, chunk, isdiag_last = work[k]
            if i == 1 and chunk is not None and chunk[:1] == [0] and h + 1 < hpc:
                # early in head h: prefetch head h+1's tensors
                emit_loads(h + 1)
            qs = head_tiles[h][0][i][:]
            st = st_pool.tile([P, 3, MW], F32, tag="st")
            pt = pt_pool.tile([P, 3, MW], F32R, tag="pt")
            if chunk is not None:
                for idx, j in enumerate(chunk):
                    nc.tensor.matmul(
                        st[:, idx, :], lhsT=kt_blk(h, j),
                        rhs=qs, start=True, stop=True,
                    )
                nc.scalar.activation(
                    pt[:, : len(chunk), :], st[:, : len(chunk), :], EXP
                )
            else:
                jb = 4 * i
                nc.tensor.matmul(st[:, 0, :], lhsT=kt_blk(h, jb),
                                 rhs=qs, start=True, stop=True)
                nc.tensor.matmul(st[:, 1, 0:256], lhsT=kt_blk(h, jb + 2),
                                 rhs=qs[:, 256:512], start=True, stop=True)
                nc.tensor.matmul(st[:, 1, 256:512], lhsT=kt_blk(h, jb + 3),
                                 rhs=qs[:, 256:512], start=True, stop=True)
                nc.tensor.matmul(st[:, 2, 0:384], lhsT=kt_blk(h, jb + 1),
                                 rhs=qs[:, 128:512], start=True, stop=True)
                st_flat = st.rearrange("p a b -> p (a b)")
                pt_flat = pt.rearrange("p a b -> p (a b)")
                nc.scalar.activation(pt_flat[:, 0:1408], st_flat[:, 0:1408], EXP)
            pend[k] = (st, pt)

        def finish(k):
            h, i, chunk, islast = work[k]
            st, pt = pend.pop(k)
            u = ustate.get((h, i))
            if u is None:
                ot_tile = otp_pool.tile([P, MW], F32, tag="otp", name="ot_tile")
                u = ustate[(h, i)] = {"ot": ot_tile, "first": True}
            ot = u["ot"]

            def pv(j, rhs, osl, stop=False):
                nc.tensor.matmul(osl, lhsT=v_blk(h, j), rhs=rhs,
                                 start=u["first"], stop=stop)
                u["first"] = False

            if chunk is not None:
                for idx, j in enumerate(chunk):
                    pv(j, pt[:, idx, :], ot[:, :])
            else:
                jb = 4 * i
                nc.vector.tensor_mul(pt[:, 0, 0:P], pt[:, 0, 0:P], tri)
                nc.vector.tensor_mul(pt[:, 2, 0:P], pt[:, 2, 0:P], tri)
                nc.vector.tensor_mul(pt[:, 1, 0:P], pt[:, 1, 0:P], tri)
                nc.vector.tensor_mul(pt[:, 1, 256:512], pt[:, 1, 256:512], ztri)
                pv(jb + 0, pt[:, 0, :], ot[:, :])
                pv(jb + 1, pt[:, 2, 0:384], ot[:, 128:512])
                pv(jb + 2, pt[:, 1, 0:256], ot[:, 256:512])
                pv(jb + 3, pt[:, 1, 256:512], ot[:, 256:512], stop=True)
                # close out the supertile
                out_sb = osb_pool.tile([P, MW], F32, tag="osb")
                nc.vector.tensor_copy(out_sb[:], ot[:])
                nc.gpsimd.dma_start(out[h, :, i * MW : (i + 1) * MW], out_sb[:])

        emit_loads(0)
        st_exp(0)
        if len(work) > 1:
            st_exp(1)
        for k in range(len(work)):
            if k + 2 < len(work):
                st_exp(k + 2)
            finish(k)


def _build(hpc=HPC, n=N):
    nc = bacc.Bacc(
        "TRN2", target_bir_lowering=False, debug=False, num_devices=NCORES
    )
    qt = nc.dram_tensor("qt", [hpc, P, n], F32R, kind="ExternalInput").ap()
    kt = nc.dram_tensor("kt", [hpc, P, n], F32R, kind="ExternalInput").ap()
    v = nc.dram_tensor("v", [hpc, n, P], F32R, kind="ExternalInput").ap()
    cmask = nc.dram_tensor("cmask", [P, 2 * P], F32R, kind="ExternalInput").ap()
    out = nc.dram_tensor("out", [hpc, P, n], F32, kind="ExternalOutput").ap()
    with tile.TileContext(nc) as tc:
        _emit_body(tc, qt, kt, v, cmask, out, hpc, n)
    nc.compile()
    return nc


_NC_CACHE = {}


def _get_nc():
    if "nc" not in _NC_CACHE:
        _NC_CACHE["nc"] = _build()
    return _NC_CACHE["nc"]


def _make_mask():
    # cmask [P, 256] = [zeros(128) | tri(128)], tri[p, c] = 1 where c >= p
    z = np.zeros((P, P), dtype=np.float32)
    c = np.arange(P)[None, :]
    p = np.arange(P)[:, None]
    tri = (c >= p).astype(np.float32)
    return np.concatenate([z, tri], axis=1)


def _prep(q, k, v):
    """Host-side reshaping/folding. Returns per-core in_maps and eq for post."""
    q = np.asarray(q, dtype=np.float32).reshape(B * H, N, D)
    k = np.asarray(k, dtype=np.float32).reshape(B * H, N, D)
    v = np.asarray(v, dtype=np.float32).reshape(B * H, N, D)

    qT = np.ascontiguousarray(q.transpose(0, 2, 1)) * np.float32(2.0 * SM)
    kT = np.ascontiguousarray(k.transpose(0, 2, 1))
    ek = np.exp(np.float32(-SM) * np.einsum("hnd,hnd->hn", k, k)).astype(np.float32)
    eq = np.exp(np.float32(-SM) * np.einsum("hnd,hnd->hn", q, q)).astype(np.float32)
    vs = (v * ek[:, :, None]).astype(np.float32)

    mask = _make_mask()
    in_maps = []
    for c in range(NCORES):
        s = slice(c * HPC, (c + 1) * HPC)
        in_maps.append(
            {
                "qt": np.ascontiguousarray(qT[s]),
                "kt": np.ascontiguousarray(kT[s]),
                "v": np.ascontiguousarray(vs[s]),
                "cmask": mask,
            }
        )
    return in_maps, eq


def _run(in_maps, trace=False):
    nc = _get_nc()
    res = bass_utils.run_bass_kernel_spmd(
        nc, in_maps, core_ids=list(range(NCORES)), trace=trace
    )
    return res


def _post(res_list, eq):
    # res_list: per-core dicts with "out" [HPC, 128(d), N(m)]
    ot = np.concatenate([r["out"] for r in res_list], axis=0)  # [B*H, D, N]
    o = ot.transpose(0, 2, 1) * eq[:, :, None]  # [B*H, N, D]
    return np.ascontiguousarray(o.reshape(B, H, N, D).astype(np.float32))


def kernel(q, k, v):
    in_maps, eq = _prep(q, k, v)
    last_err = None
    for attempt in range(3):
        try:
            res = _run(in_maps, trace=False)
            return _post(res.results, eq)
        except Exception as e:  # axon/NRT first-run flakiness: retry
            last_err = e
            time.sleep(2.0)
    raise last_err
